# revision 2
# baseline (speedup 1.0000x reference)
"""Squeeze-and-Excitation attention module on 8 Trainium2 NeuronCores.

Reference computation (per image b):
    y[c]  = mean(x[b, c, :, :])                      # global average pool
    z     = relu(w1 @ y + b1)                        # FC 512 -> 32
    s     = sigmoid(w2 @ z + b2)                     # FC 32 -> 512
    out[b, c, :, :] = x[b, c, :, :] * s[c]

Sharding: data-parallel over batch. 32 images / 8 cores = 4 images per
core; the tiny FC weights are replicated.

The kernel is HBM-bandwidth-bound, so x and out travel as bf16 (host
casts f32 -> bf16 before upload and back after download): 16 MB in +
16 MB out per core instead of 64 MB, i.e. ~75 us of DMA at the ~430
GB/s per-core fabric ceiling. bf16 rounding on x and out contributes
~1.6e-3 relative error, well inside the 2e-2 gate; the FC path stays
f32 (pool sums accumulate to f32, weights f32, sigmoid output cast to
bf16 only for the broadcast multiply).

Per-core layout: one [128, 4, 4096] bf16 tile per image -- all four
images resident in SBUF at once (128 KB of the 208 KB per partition),
so loads never wait on stores. Channel c = 4-way chunked: chunk k =
c // 128 lives at free-dim slot k, partition c % 128.

Engine plan:
    sync (HWDGE)   4 x 4 MB image loads, issued up front
    gpsimd (SWDGE) weight loads, then 16 x 1 MB chunk stores
    DVE            per image: one [128,4,4096] reduce (f32 sums), then
                   the 4 in-place bf16 broadcast multiplies (4x mode,
                   ~1.1 us/chunk vs 3.7 on ACT)
    PE             4+4 tiny matmuls per image (FC1/FC2, f32, PSUM)
    ACT            relu (with 1/HW prescale) + 4 sigmoids per image

Weights layouts (host-prepared):
    w1t    [128, 4, 32]    w1t[p, k, r] = w1[r, 128k + p]
    b1     [32, 1]
    w2t    [32, 4, 128]    w2t[r, k, p] = w2[128k + p, r]
    b2c    [128, 4]        b2c[p, k]   = b2[128k + p]
"""

import numpy as np

B = 32
C = 512
HW = 64 * 64
N_CORES = 8
B_LOC = B // N_CORES
KC = C // 128  # channel chunks of 128

_NC_CACHE = {}

# Set by test harness to capture a profile; harmless default for grading.
TRACE = False
LAST_RESULT = None


def _build_nc():
    from contextlib import ExitStack

    import concourse.tile as tile
    from concourse import bacc, mybir

    f32 = mybir.dt.float32
    bf16 = mybir.dt.bfloat16
    AF = mybir.ActivationFunctionType
    nc = bacc.Bacc("TRN2", target_bir_lowering=False, debug=False)

    x = nc.dram_tensor("x", [B_LOC, 128, KC, HW], bf16, kind="ExternalInput")
    w1t = nc.dram_tensor("w1t", [128, KC, 32], f32, kind="ExternalInput")
    b1 = nc.dram_tensor("b1", [32, 1], f32, kind="ExternalInput")
    w2t = nc.dram_tensor("w2t", [32, KC, 128], f32, kind="ExternalInput")
    b2c = nc.dram_tensor("b2c", [128, KC], f32, kind="ExternalInput")
    out = nc.dram_tensor("out", [B_LOC, 128, KC, HW], bf16, kind="ExternalOutput")

    with ExitStack() as ctx:
        tc = ctx.enter_context(tile.TileContext(nc))
        singles = ctx.enter_context(tc.tile_pool(name="singles", bufs=1))
        xpool = ctx.enter_context(tc.tile_pool(name="xpool", bufs=B_LOC))
        small = ctx.enter_context(tc.tile_pool(name="small", bufs=2))
        psum = ctx.enter_context(tc.tile_pool(name="psum", bufs=2, space="PSUM"))

        w1t_sb = singles.tile([128, KC, 32], f32)
        b1_sb = singles.tile([32, 1], f32)
        w2t_sb = singles.tile([32, KC, 128], f32)
        b2_sb = singles.tile([128, KC], f32)

        xts = []
        for b in range(B_LOC):
            xt = xpool.tile([128, KC, HW], bf16, tag="x")
            nc.sync.dma_start(out=xt, in_=x[b])
            xts.append(xt)
            if b == 0:
                # Weight loads ride the otherwise-idle SWDGE queue so
                # they never delay image loads on the Sync ring.
                nc.gpsimd.dma_start(out=w1t_sb, in_=w1t[:])
                nc.gpsimd.dma_start(out=b1_sb, in_=b1[:])
                nc.gpsimd.dma_start(out=w2t_sb, in_=w2t[:])
                nc.gpsimd.dma_start(out=b2_sb, in_=b2c[:])

        for b in range(B_LOC):
            xt = xts[b]
            sums = small.tile([128, KC], f32, tag="sums")
            nc.vector.tensor_reduce(
                out=sums,
                in_=xt,
                axis=mybir.AxisListType.X,
                op=mybir.AluOpType.add,
            )

            zp = psum.tile([32, 1], f32, tag="z")
            for k in range(KC):
                nc.tensor.matmul(
                    zp,
                    lhsT=w1t_sb[:, k, :],
                    rhs=sums[:, k : k + 1],
                    start=(k == 0),
                    stop=(k == KC - 1),
                )
            z = small.tile([32, 1], f32, tag="z_sb")
            nc.scalar.activation(z, zp, AF.Relu, bias=b1_sb, scale=1.0 / HW)

            sp = psum.tile([128, KC], f32, tag="s")
            for k in range(KC):
                nc.tensor.matmul(
                    sp[:, k : k + 1],
                    lhsT=w2t_sb[:, k, :],
                    rhs=z,
                    start=True,
                    stop=True,
                )
            s_tiles = []
            for k in range(KC):
                s = small.tile([128, 1], bf16, tag=f"s{k}")
                nc.scalar.activation(
                    s, sp[:, k : k + 1], AF.Sigmoid, bias=b2_sb[:, k : k + 1]
                )
                s_tiles.append(s)

            # In-place broadcast multiply on DVE (bf16 4x mode), then
            # store each chunk as soon as its multiply lands. Stores
            # ride the SWDGE queue so a store waiting on compute never
            # head-of-line-blocks the Sync load ring.
            for k in range(KC):
                nc.vector.tensor_scalar_mul(xt[:, k], xt[:, k], s_tiles[k])
                nc.gpsimd.dma_start(out=out[b, :, k], in_=xt[:, k])

    nc.compile()
    return nc


def _get_nc():
    if "nc" not in _NC_CACHE:
        _NC_CACHE["nc"] = _build_nc()
    return _NC_CACHE["nc"]


def kernel(x, w1, b1, w2, b2):
    global LAST_RESULT
    import ml_dtypes
    from concourse.bass_utils import run_bass_kernel_spmd

    bf16 = ml_dtypes.bfloat16
    # [B, C, 64, 64] f32 -> [B, 128, KC, HW] bf16 (partition-major chunks)
    xb = np.ascontiguousarray(
        x.reshape(B, KC, 128, HW).astype(bf16).transpose(0, 2, 1, 3)
    )
    w1t = np.ascontiguousarray(w1.reshape(32, KC, 128).transpose(2, 1, 0))
    b1c = np.ascontiguousarray(b1.reshape(32, 1))
    w2t = np.ascontiguousarray(w2.reshape(KC, 128, 32).transpose(2, 0, 1))
    b2c = np.ascontiguousarray(b2.reshape(KC, 128).T)

    in_maps = [
        {
            "x": np.ascontiguousarray(xb[i * B_LOC : (i + 1) * B_LOC]),
            "w1t": w1t,
            "b1": b1c,
            "w2t": w2t,
            "b2c": b2c,
        }
        for i in range(N_CORES)
    ]

    nc = _get_nc()
    res = run_bass_kernel_spmd(
        nc, in_maps, core_ids=list(range(N_CORES)), trace=TRACE
    )
    LAST_RESULT = res
    out = np.concatenate([r["out"] for r in res.results], axis=0)
    # [B, 128, KC, HW] bf16 -> [B, C, 64, 64] f32
    return np.ascontiguousarray(
        out.transpose(0, 2, 1, 3).reshape(B, C, 64, 64)
    ).astype(np.float32)


# revision 3
# speedup vs baseline: 1.5707x; 1.5707x over previous
"""Squeeze-and-Excitation attention module on 8 Trainium2 NeuronCores.

Reference computation (per image b):
    y[c]  = mean(x[b, c, :, :])                      # global average pool
    z     = relu(w1 @ y + b1)                        # FC 512 -> 32
    s     = sigmoid(w2 @ z + b2)                     # FC 32 -> 512
    out[b, c, :, :] = x[b, c, :, :] * s[c]

Sharding: data-parallel over batch. 32 images / 8 cores = 4 images per
core; the tiny FC weights are replicated.

The kernel is HBM-bandwidth-bound, so x and out travel as bf16 (host
casts f32 -> bf16 before upload and back after download): 16 MB in +
16 MB out per core instead of 64 MB, i.e. ~75 us of DMA at the ~430
GB/s per-core fabric ceiling. bf16 rounding on x and out contributes
~1.6e-3 relative error, well inside the 2e-2 gate; the FC path stays
f32 (pool sums accumulate to f32, weights f32, sigmoid output cast to
bf16 only for the broadcast multiply).

Per-core layout: one [128, 4, 4096] bf16 tile per image -- all four
images resident in SBUF at once (128 KB of the 208 KB per partition),
so loads never wait on stores. Channel c = 4-way chunked: chunk k =
c // 128 lives at free-dim slot k, partition c % 128.

Engine plan:
    sync (HWDGE)   4 x 4 MB image loads, issued up front
    gpsimd (SWDGE) weight loads, then 16 x 1 MB chunk stores
    DVE            per image: one [128,4,4096] reduce (f32 sums), then
                   the 4 in-place bf16 broadcast multiplies (4x mode,
                   ~1.1 us/chunk vs 3.7 on ACT)
    PE             4+4 tiny matmuls per image (FC1/FC2, f32, PSUM)
    ACT            relu (with 1/HW prescale) + 4 sigmoids per image

Weights layouts (host-prepared):
    w1t    [128, 4, 32]    w1t[p, k, r] = w1[r, 128k + p]
    b1     [32, 1]
    w2t    [32, 4, 128]    w2t[r, k, p] = w2[128k + p, r]
    b2c    [128, 4]        b2c[p, k]   = b2[128k + p]
"""

import numpy as np

B = 32
C = 512
HW = 64 * 64
N_CORES = 8
B_LOC = B // N_CORES
KC = C // 128  # channel chunks of 128

_NC_CACHE = {}

# Set by test harness to capture a profile; harmless default for grading.
TRACE = False
LAST_RESULT = None


def _build_nc():
    from contextlib import ExitStack

    import concourse.tile as tile
    from concourse import bacc, mybir

    f32 = mybir.dt.float32
    bf16 = mybir.dt.bfloat16
    AF = mybir.ActivationFunctionType
    nc = bacc.Bacc("TRN2", target_bir_lowering=False, debug=False)

    x = nc.dram_tensor("x", [B_LOC, 128, KC, HW], bf16, kind="ExternalInput")
    w1t = nc.dram_tensor("w1t", [128, KC, 32], f32, kind="ExternalInput")
    b1 = nc.dram_tensor("b1", [32, 1], f32, kind="ExternalInput")
    w2t = nc.dram_tensor("w2t", [32, KC, 128], f32, kind="ExternalInput")
    b2c = nc.dram_tensor("b2c", [128, KC], f32, kind="ExternalInput")
    out = nc.dram_tensor("out", [B_LOC, 128, KC, HW], bf16, kind="ExternalOutput")

    with ExitStack() as ctx:
        tc = ctx.enter_context(tile.TileContext(nc))
        singles = ctx.enter_context(tc.tile_pool(name="singles", bufs=1))
        xpool = ctx.enter_context(tc.tile_pool(name="xpool", bufs=B_LOC))
        small = ctx.enter_context(tc.tile_pool(name="small", bufs=2))
        psum = ctx.enter_context(tc.tile_pool(name="psum", bufs=2, space="PSUM"))

        w1t_sb = singles.tile([128, KC, 32], f32)
        b1_sb = singles.tile([32, 1], f32)
        w2t_sb = singles.tile([32, KC, 128], f32)
        b2_sb = singles.tile([128, KC], f32)

        xts = []
        for b in range(B_LOC):
            xt = xpool.tile([128, KC, HW], bf16, tag="x")
            nc.sync.dma_start(out=xt, in_=x[b])
            xts.append(xt)
            if b == 0:
                # Weight loads ride the otherwise-idle SWDGE queue so
                # they never delay image loads on the Sync ring.
                nc.gpsimd.dma_start(out=w1t_sb, in_=w1t[:])
                nc.gpsimd.dma_start(out=b1_sb, in_=b1[:])
                nc.gpsimd.dma_start(out=w2t_sb, in_=w2t[:])
                nc.gpsimd.dma_start(out=b2_sb, in_=b2c[:])

        for b in range(B_LOC):
            xt = xts[b]
            sums = small.tile([128, KC], f32, tag="sums")
            nc.vector.tensor_reduce(
                out=sums,
                in_=xt,
                axis=mybir.AxisListType.X,
                op=mybir.AluOpType.add,
            )

            zp = psum.tile([32, 1], f32, tag="z")
            for k in range(KC):
                nc.tensor.matmul(
                    zp,
                    lhsT=w1t_sb[:, k, :],
                    rhs=sums[:, k : k + 1],
                    start=(k == 0),
                    stop=(k == KC - 1),
                )
            z = small.tile([32, 1], f32, tag="z_sb")
            nc.scalar.activation(z, zp, AF.Relu, bias=b1_sb, scale=1.0 / HW)

            sp = psum.tile([128, KC], f32, tag="s")
            for k in range(KC):
                nc.tensor.matmul(
                    sp[:, k : k + 1],
                    lhsT=w2t_sb[:, k, :],
                    rhs=z,
                    start=True,
                    stop=True,
                )
            s_tiles = []
            for k in range(KC):
                # f32: DVE tensor_scalar requires a float32 scalar operand.
                s = small.tile([128, 1], f32, tag=f"s{k}")
                nc.scalar.activation(
                    s, sp[:, k : k + 1], AF.Sigmoid, bias=b2_sb[:, k : k + 1]
                )
                s_tiles.append(s)

            # In-place broadcast multiply on DVE (bf16 4x mode), then
            # store each chunk as soon as its multiply lands. Stores
            # ride the SWDGE queue so a store waiting on compute never
            # head-of-line-blocks the Sync load ring.
            for k in range(KC):
                nc.vector.tensor_scalar_mul(xt[:, k], xt[:, k], s_tiles[k])
                nc.gpsimd.dma_start(out=out[b, :, k], in_=xt[:, k])

    nc.compile()
    return nc


def _get_nc():
    if "nc" not in _NC_CACHE:
        _NC_CACHE["nc"] = _build_nc()
    return _NC_CACHE["nc"]


def kernel(x, w1, b1, w2, b2):
    global LAST_RESULT
    import ml_dtypes
    from concourse.bass_utils import run_bass_kernel_spmd

    bf16 = ml_dtypes.bfloat16
    # [B, C, 64, 64] f32 -> [B, 128, KC, HW] bf16 (partition-major chunks)
    xb = np.ascontiguousarray(
        x.reshape(B, KC, 128, HW).astype(bf16).transpose(0, 2, 1, 3)
    )
    w1t = np.ascontiguousarray(w1.reshape(32, KC, 128).transpose(2, 1, 0))
    b1c = np.ascontiguousarray(b1.reshape(32, 1))
    w2t = np.ascontiguousarray(w2.reshape(KC, 128, 32).transpose(2, 0, 1))
    b2c = np.ascontiguousarray(b2.reshape(KC, 128).T)

    in_maps = [
        {
            "x": np.ascontiguousarray(xb[i * B_LOC : (i + 1) * B_LOC]),
            "w1t": w1t,
            "b1": b1c,
            "w2t": w2t,
            "b2c": b2c,
        }
        for i in range(N_CORES)
    ]

    nc = _get_nc()
    res = run_bass_kernel_spmd(
        nc, in_maps, core_ids=list(range(N_CORES)), trace=TRACE
    )
    LAST_RESULT = res
    out = np.concatenate([r["out"] for r in res.results], axis=0)
    # [B, 128, KC, HW] bf16 -> [B, C, 64, 64] f32
    return np.ascontiguousarray(
        out.transpose(0, 2, 1, 3).reshape(B, C, 64, 64)
    ).astype(np.float32)
